# revision 19
# baseline (speedup 1.0000x reference)
"""BPCA pooling layer on 8 Trainium2 NeuronCores (Bass/Tile).

Math: per sample, the reference's `data = patches.reshape(-1, 4)` groups 4
consecutive channels (C=256 is divisible by 4), so `data` is exactly the
sample's contiguous buffer viewed as [N, 4] with N = H*W*C/4.  The layer is:

  1. per-column mean/std over N rows, dn = (data-mean)/std
  2. gram = dn^T dn (4x4), comp = top eigenvector (jnp.linalg.eigh)
  3. out = (dn @ comp) reshaped to [H/2, W/2, C] with channel permutation
     c' = (2*di+dj)*64 + (c//4)

Device plan (2 samples per core, pure data parallel):
  pass 1: PE computes the full 256x256 channel second-moment matrix
          M[c,c'] = sum_pix x[pix,c]*x[pix,c'] plus channel sums (ones
          column), accumulated in PSUM over all pixels.  float32r matmuls
          (1 col/cycle at N>=256) keep PE under the DMA roofline.
  host:   fold M into the 4x4 gram (S_kl = sum_g M[4g+k,4g+l]), compute
          mean/std/gram in f64, eigh on CPU jax (same implementation the
          reference uses), derive w_k = comp_k/std_k and
          bias = -sum_k mean_k*comp_k/std_k.
  pass 2: out = sum_k x_k*w_k + bias -- 4 fused scalar_tensor_tensor /
          activation ops per tile on DVE+ACT, streaming at the DMA roofline,
          with the output channel permutation folded into the DMA pattern.
"""

import numpy as np

# ---------------------------------------------------------------------------
# Problem constants (hardcoded per spec)
# ---------------------------------------------------------------------------
B, H, W, C = 16, 112, 112, 256
N_CORES = 8
SPC = B // N_CORES          # samples per core = 2
PIX = H * W                 # 12544 pixels per sample
NBLK = PIX // 128           # 98 pixel-blocks of 128
BT = 7                      # pass-1 big tiles per sample
BLK_PER_BT = NBLK // BT     # 14 blocks per big tile
BSTRIDE = 258               # per-block SBUF cols: 256 data + 1 ones + 1 pad
NROWS = PIX * C // 4        # 802816 rows of the [N, 4] data matrix
HO, WO = H // 2, W // 2     # 56 x 56 output
T2 = 14                     # pass-2 tiles per sample (4 output rows each)
HPT = HO // T2              # 4 output rows per pass-2 tile

_programs = None
LAST_PROFILE = {}
TRACE = False
TRACE_DIRS = {}


# ---------------------------------------------------------------------------
# TileContext with a walrus-compatible tail drain
# ---------------------------------------------------------------------------
def _make_tile_context(nc):
    from concourse.tile import TileContext
    return TileContext(nc)


def _split_sync_waits(nc):
    """walrus (CoreV2/V3 codegen) rejects instructions carrying more than 2
    sync commands (waits + updates combined); Tile freely emits e.g. 2 waits
    + 1 update.  Hoist excess waits onto same-engine NOPs inserted directly
    before the offending instruction -- same engine means the same program-
    order point, so semantics are unchanged."""
    import concourse.mybir as mybir

    def mint_nop(engine):
        inner = nc.engines[engine].nop().ins
        for blk in nc.m.functions[0].blocks:
            il = blk.instructions
            for k in range(len(il) - 1, -1, -1):
                if il[k] is inner:
                    il.pop(k)
                    return inner
        raise RuntimeError("minted nop not found in any block")

    for fn in nc.m.functions:
        for blk in fn.blocks:
            il = blk.instructions
            i = 0
            while i < len(il):
                inst = il[i]
                si = inst.sync_info
                waits = list(si.on_wait) if si and si.on_wait else []
                upds = list(si.on_update) if si and si.on_update else []
                # observed walrus limits: at most 1 wait per instruction
                # (1 wait + 1 update compiles; 2 waits anywhere does not)
                if len(waits) > 1:
                    extra, keep = waits[:-1], waits[-1:]
                    for wchunk in extra:
                        nop = mint_nop(inst.engine)
                        nop.sync_info = mybir.SyncInfo(
                            on_wait=[wchunk], on_update=[])
                        il.insert(i, nop)
                        i += 1
                    inst.sync_info = mybir.SyncInfo(
                        on_wait=keep, on_update=upds)
                i += 1


def _build_pass1():
    import concourse.bass as bass
    import concourse.mybir as mybir

    f32 = mybir.dt.float32
    f32r = mybir.dt.float32r

    nc = bass.Bass("TRN2", target_bir_lowering=False, debug=False,
                   num_devices=N_CORES)
    # float32r: same bits as f32 (np dtype float32); typing the whole
    # producer chain f32r satisfies walrus's checkMatmultFP32r while the
    # PE runs the matmuls at 1 col/cycle (vs 4 for plain fp32).
    # The host pre-interleaves a ones column per block (col 256 of each
    # 258-wide block) so one DMA loads data + ones and no on-device memset
    # is needed.
    x = nc.dram_tensor("x", [SPC, BT, BLK_PER_BT, 128, BSTRIDE], f32r,
                       kind="ExternalInput").ap()
    stats = nc.dram_tensor("stats", [SPC, 2, 128, 257], f32,
                           kind="ExternalOutput").ap()

    with _make_tile_context(nc) as tc:
        with (
            tc.tile_pool(name="inp", bufs=3) as inp,
            tc.tile_pool(name="psum", bufs=2, space="PSUM") as psum,
            tc.tile_pool(name="sout", bufs=2) as soutp,
        ):
            for s in range(SPC):
                # fp32r matmuls need an even moving free size -> 258
                ps1 = psum.tile([128, BSTRIDE], f32, tag="ps1")
                ps2 = psum.tile([128, BSTRIDE], f32, tag="ps2")
                for bt in range(BT):
                    t = inp.tile([128, BLK_PER_BT * BSTRIDE], f32r)
                    t3 = t[:].rearrange("p (j b) -> p j b", b=BSTRIDE)
                    nc.sync.dma_start(
                        out=t3,
                        in_=x[s, bt].rearrange("j p c -> p j c"),
                    )
                    for j in range(BLK_PER_BT):
                        first = bt == 0 and j == 0
                        last = bt == BT - 1 and j == BLK_PER_BT - 1
                        rhs = t3[:, j:j + 1, 0:BSTRIDE]
                        lh1 = t3[:, j:j + 1, 0:128]
                        lh2 = t3[:, j:j + 1, 128:256]
                        nc.tensor.matmul(ps1[:, 0:BSTRIDE], lh1, rhs,
                                         start=first, stop=last,
                                         skip_group_check=True)
                        nc.tensor.matmul(ps2[:, 0:BSTRIDE], lh2, rhs,
                                         start=first, stop=last,
                                         skip_group_check=True)
                so = soutp.tile([128, 514], f32)
                nc.vector.tensor_copy(out=so[:, 0:257], in_=ps1[:, 0:257])
                nc.vector.tensor_copy(out=so[:, 257:514], in_=ps2[:, 0:257])
                nc.sync.dma_start(
                    out=stats[s].rearrange("t p c -> p t c"),
                    in_=so[:].rearrange("p (t c) -> p t c", c=257),
                )
    _split_sync_waits(nc)
    return nc


def _build_pass2():
    import concourse.bass as bass
    import concourse.mybir as mybir

    f32 = mybir.dt.float32
    alu = mybir.AluOpType
    T = 14  # tiles per sample: 4 output rows (8 input rows) each

    nc = bass.Bass("TRN2", target_bir_lowering=False, debug=False,
                   num_devices=N_CORES)
    # x viewed as [s, t, ohi, di, wi, dj, c]:
    #   input row h = 8t + 2*ohi + di, input col w = 2*wi + dj
    x = nc.dram_tensor("x", [SPC, T, 4, 2, WO, 2, C], f32,
                       kind="ExternalInput").ap()
    wb = nc.dram_tensor("wb", [SPC, 128, 8], f32, kind="ExternalInput").ap()
    # out viewed as [s, t, outpix, c']: outpix = ohi*56 + wi
    out = nc.dram_tensor("out", [SPC, T, 224, C], f32,
                         kind="ExternalOutput").ap()

    # Whole chain on DVE: same-engine deps cost no semaphore waits, keeping
    # every instruction at <=2 sync waits (walrus CoreV3 limit).
    with _make_tile_context(nc) as tc:
        with (
            tc.tile_pool(name="w", bufs=1) as wpool,
            tc.tile_pool(name="inp", bufs=4) as inp,
            tc.tile_pool(name="acc", bufs=3) as accp,
        ):
            for s in range(SPC):
                wt = wpool.tile([128, 8], f32, tag=f"wb{s}")
                nc.sync.dma_start(out=wt[:], in_=wb[s])
                # touch wb on DVE so later DVE ops inherit the dep by
                # program order instead of each carrying a sem wait
                wl = wpool.tile([128, 8], f32, tag=f"wl{s}")
                nc.vector.tensor_copy(out=wl[:], in_=wt[:])
                w0 = wl[0:112, 0:1]
                w1 = wl[0:112, 1:2]
                w2 = wl[0:112, 2:3]
                w3 = wl[0:112, 3:4]
                bias = wl[0:112, 4:5]
                for t in range(T):
                    # partition = output-pixel pair (o = 2p + oslot);
                    # free = (oslot, di, dj, c): quarter q = 2*di + dj
                    # matches the output channel-block order
                    it = inp.tile([112, 2 * 4 * C], f32)
                    it5 = it[:].rearrange("p (o di dj c) -> p o di dj c",
                                          o=2, di=2, dj=2)
                    for di in range(2):
                        nc.sync.dma_start(
                            out=it5[:, :, di],
                            in_=x[s, t, :, di].rearrange(
                                "ohi wi dj c -> ohi (wi dj c)"),
                        )
                    # absorb the first DMA's completion into DVE program
                    # order so the compute ops below carry <=1 sem wait
                    sc = accp.tile([112, 4], f32, tag="sc", bufs=1)
                    nc.vector.tensor_copy(out=sc[:], in_=it[:, 0:4])
                    it6 = it[:].rearrange("p (o q j k) -> p o q j k",
                                          o=2, q=4, k=4)
                    a0 = accp.tile([112, 2 * C], f32, tag="a0")
                    a1 = accp.tile([112, 2 * C], f32, tag="a1")
                    a2 = accp.tile([112, 2 * C], f32, tag="a2")
                    ot = accp.tile([112, 2 * C], f32, tag="ot")

                    def v(tt):
                        return tt[:].rearrange("p (o q j) -> p o q j",
                                               o=2, q=4)[:, :, :, :, None]

                    # a0 = x0*w0 + bias; a_k = x_k*w_k + a_{k-1}
                    nc.vector.tensor_scalar(
                        v(a0), it6[:, :, :, :, 0:1], w0, bias,
                        op0=alu.mult, op1=alu.add)
                    nc.vector.scalar_tensor_tensor(
                        v(a1), it6[:, :, :, :, 1:2], w1, v(a0),
                        op0=alu.mult, op1=alu.add)
                    nc.vector.scalar_tensor_tensor(
                        v(a2), it6[:, :, :, :, 2:3], w2, v(a1),
                        op0=alu.mult, op1=alu.add)
                    nc.vector.scalar_tensor_tensor(
                        v(ot), it6[:, :, :, :, 3:4], w3, v(a2),
                        op0=alu.mult, op1=alu.add)
                    nc.sync.dma_start(
                        out=out[s, t],
                        in_=ot[:].rearrange("p (o c) -> p o c", o=2),
                    )
    _split_sync_waits(nc)
    return nc


def _get_programs():
    global _programs
    if _programs is None:
        _programs = (_build_pass1(), _build_pass2())
    return _programs


def _host_middle(stats):
    """stats: [B, 2, 128, 257] f32 -> w [B, 4] f64, bias [B] f64.

    Follows the reference downstream exactly: gram from (S - N mu mu^T) /
    (sigma sigma^T), comp = eigh(gram f32) top eigenvector on CPU jax.
    """
    stats = stats.astype(np.float64)
    M = np.concatenate([stats[:, 0, :, :256], stats[:, 1, :, :256]], axis=1)
    chansum = np.concatenate([stats[:, 0, :, 256], stats[:, 1, :, 256]], axis=1)

    # fold channels c = 4g+k into columns k
    Mg = M.reshape(B, 64, 4, 64, 4)
    S = np.einsum("bgkgl->bkl", Mg)                      # [B, 4, 4]
    colsum = chansum.reshape(B, 64, 4).sum(axis=1)       # [B, 4]

    mu = colsum / NROWS
    e2 = np.einsum("bkk->bk", S) / NROWS
    var = np.maximum(e2 - mu * mu, 0.0)
    sigma = np.sqrt(var)
    denom = sigma[:, :, None] * sigma[:, None, :]
    gram = (S - NROWS * mu[:, :, None] * mu[:, None, :])
    with np.errstate(divide="ignore", invalid="ignore"):
        gram = np.where(denom > 0, gram / np.where(denom > 0, denom, 1.0), 0.0)

    # eigh with the same implementation/backend the reference uses (CPU jax)
    import jax
    import jax.numpy as jnp
    with jax.default_device(jax.devices("cpu")[0]):
        V = np.asarray(jnp.linalg.eigh(jnp.asarray(gram, jnp.float32))[1])
    comp = V[:, :, -1].astype(np.float64)                # top eigenvector

    with np.errstate(divide="ignore", invalid="ignore"):
        w = np.where(sigma > 0, comp / np.where(sigma > 0, sigma, 1.0), 0.0)
    bias = -(mu * w).sum(axis=1)
    return w, bias


def kernel(x):
    from concourse.bass_utils import run_bass_kernel_spmd

    x = np.ascontiguousarray(np.asarray(x), dtype=np.float32)
    assert x.shape == (B, H, W, C), x.shape
    nc1, nc2 = _get_programs()
    core_ids = list(range(N_CORES))

    shards = [x[c * SPC:(c + 1) * SPC] for c in range(N_CORES)]

    # pass-1 input: data blocks padded to 258 cols with a ones column at 256
    xp = np.zeros((B, BT, BLK_PER_BT, 128, BSTRIDE), np.float32)
    xp[..., :C] = x.reshape(B, BT, BLK_PER_BT, 128, C)
    xp[..., C] = 1.0
    in1 = [{"x": xp[c * SPC:(c + 1) * SPC]} for c in range(N_CORES)]
    kw1 = dict(trace=True, tmpdir=TRACE_DIRS.get("pass1")) if TRACE else {}
    r1 = run_bass_kernel_spmd(nc1, in1, core_ids, **kw1)
    if TRACE:
        LAST_PROFILE["pass1_ns"] = r1.exec_time_ns
    stats = np.concatenate([r1.results[c]["stats"] for c in range(N_CORES)])

    w, bias = _host_middle(stats)
    wbs = []
    for c in range(N_CORES):
        a = np.zeros((SPC, 128, 8), np.float32)
        for s in range(SPC):
            b = c * SPC + s
            a[s, :, 0:4] = w[b].astype(np.float32)
            a[s, :, 4] = np.float32(bias[b])
        wbs.append(a)

    in2 = [{"x": shards[c].reshape(SPC, 14, 4, 2, WO, 2, C),
            "wb": wbs[c]} for c in range(N_CORES)]
    kw2 = dict(trace=True, tmpdir=TRACE_DIRS.get("pass2")) if TRACE else {}
    r2 = run_bass_kernel_spmd(nc2, in2, core_ids, **kw2)
    if TRACE:
        LAST_PROFILE["pass2_ns"] = r2.exec_time_ns

    out = np.concatenate([r2.results[c]["out"] for c in range(N_CORES)])
    # [B, 28, 112, C] -> [B, 56, 56, C]: tile t holds output rows 2t, 2t+1
    return out.reshape(B, HO, WO, C)


# revision 24
# speedup vs baseline: 1.3956x; 1.3956x over previous
"""BPCA pooling layer on 8 Trainium2 NeuronCores (Bass/Tile).

Math: per sample, the reference's `data = patches.reshape(-1, 4)` groups 4
consecutive channels (C=256 is divisible by 4), so `data` is exactly the
sample's contiguous buffer viewed as [N, 4] with N = H*W*C/4.  The layer is:

  1. per-column mean/std over N rows, dn = (data-mean)/std
  2. gram = dn^T dn (4x4), comp = top eigenvector (jnp.linalg.eigh)
  3. out = (dn @ comp) reshaped to [H/2, W/2, C] with channel permutation
     c' = (2*di+dj)*64 + (c//4)

Device plan (2 samples per core, pure data parallel):
  pass 1: PE computes the full 256x256 channel second-moment matrix
          M[c,c'] = sum_pix x[pix,c]*x[pix,c'] plus channel sums (ones
          column), accumulated in PSUM over all pixels.  float32r matmuls
          (1 col/cycle at N>=256) keep PE under the DMA roofline.
  host:   fold M into the 4x4 gram (S_kl = sum_g M[4g+k,4g+l]), compute
          mean/std/gram in f64, eigh on CPU jax (same implementation the
          reference uses), derive w_k = comp_k/std_k and
          bias = -sum_k mean_k*comp_k/std_k.
  pass 2: out = sum_k x_k*w_k + bias -- 4 fused scalar_tensor_tensor /
          activation ops per tile on DVE+ACT, streaming at the DMA roofline,
          with the output channel permutation folded into the DMA pattern.
"""

import numpy as np

# ---------------------------------------------------------------------------
# Problem constants (hardcoded per spec)
# ---------------------------------------------------------------------------
B, H, W, C = 16, 112, 112, 256
N_CORES = 8
SPC = B // N_CORES          # samples per core = 2
PIX = H * W                 # 12544 pixels per sample
NBLK = PIX // 128           # 98 pixel-blocks of 128
BT = 7                      # pass-1 big tiles per sample
BLK_PER_BT = NBLK // BT     # 14 blocks per big tile
BSTRIDE = 258               # per-block SBUF cols: 256 data + 1 ones + 1 pad
NROWS = PIX * C // 4        # 802816 rows of the [N, 4] data matrix
HO, WO = H // 2, W // 2     # 56 x 56 output
T2 = 14                     # pass-2 tiles per sample (4 output rows each)
HPT = HO // T2              # 4 output rows per pass-2 tile

_programs = None
LAST_PROFILE = {}
TRACE = False
TRACE_DIRS = {}


# ---------------------------------------------------------------------------
# TileContext with a walrus-compatible tail drain
# ---------------------------------------------------------------------------
def _make_tile_context(nc):
    from concourse.tile import TileContext
    return TileContext(nc)


def _split_sync_waits(nc):
    """walrus (CoreV2/V3 codegen) rejects instructions carrying more than 2
    sync commands (waits + updates combined); Tile freely emits e.g. 2 waits
    + 1 update.  Hoist excess waits onto same-engine NOPs inserted directly
    before the offending instruction -- same engine means the same program-
    order point, so semantics are unchanged."""
    import concourse.mybir as mybir

    def mint_nop(engine):
        inner = nc.engines[engine].nop().ins
        for blk in nc.m.functions[0].blocks:
            il = blk.instructions
            for k in range(len(il) - 1, -1, -1):
                if il[k] is inner:
                    il.pop(k)
                    return inner
        raise RuntimeError("minted nop not found in any block")

    for fn in nc.m.functions:
        for blk in fn.blocks:
            il = blk.instructions
            i = 0
            while i < len(il):
                inst = il[i]
                si = inst.sync_info
                waits = list(si.on_wait) if si and si.on_wait else []
                upds = list(si.on_update) if si and si.on_update else []
                # observed walrus limits: at most 1 wait per instruction
                # (1 wait + 1 update compiles; 2 waits anywhere does not)
                if len(waits) > 1:
                    extra, keep = waits[:-1], waits[-1:]
                    for wchunk in extra:
                        nop = mint_nop(inst.engine)
                        nop.sync_info = mybir.SyncInfo(
                            on_wait=[wchunk], on_update=[])
                        il.insert(i, nop)
                        i += 1
                    inst.sync_info = mybir.SyncInfo(
                        on_wait=keep, on_update=upds)
                i += 1


def _build_pass1():
    import concourse.bass as bass
    import concourse.mybir as mybir

    f32 = mybir.dt.float32
    f32r = mybir.dt.float32r

    nc = bass.Bass("TRN2", target_bir_lowering=False, debug=False,
                   num_devices=N_CORES)
    # float32r: same bits as f32 (np dtype float32); typing the whole
    # producer chain f32r satisfies walrus's checkMatmultFP32r while the
    # PE runs the matmuls at 1 col/cycle (vs 4 for plain fp32).
    # The host pre-interleaves a ones column per block (col 256 of each
    # 258-wide block) so one DMA loads data + ones and no on-device memset
    # is needed.
    x = nc.dram_tensor("x", [SPC, BT, BLK_PER_BT, 128, BSTRIDE], f32r,
                       kind="ExternalInput").ap()
    stats = nc.dram_tensor("stats", [SPC, 2, 128, 257], f32,
                           kind="ExternalOutput").ap()

    with _make_tile_context(nc) as tc:
        with (
            tc.tile_pool(name="inp", bufs=3) as inp,
            tc.tile_pool(name="psum", bufs=2, space="PSUM") as psum,
            tc.tile_pool(name="sout", bufs=2) as soutp,
        ):
            for s in range(SPC):
                # fp32r matmuls need an even moving free size -> 258
                ps1 = psum.tile([128, BSTRIDE], f32, tag="ps1")
                ps2 = psum.tile([128, BSTRIDE], f32, tag="ps2")
                for bt in range(BT):
                    t = inp.tile([128, BLK_PER_BT * BSTRIDE], f32r)
                    t3 = t[:].rearrange("p (j b) -> p j b", b=BSTRIDE)
                    nc.sync.dma_start(
                        out=t3,
                        in_=x[s, bt].rearrange("j p c -> p j c"),
                    )
                    for j in range(BLK_PER_BT):
                        first = bt == 0 and j == 0
                        last = bt == BT - 1 and j == BLK_PER_BT - 1
                        rhs = t3[:, j:j + 1, 0:BSTRIDE]
                        lh1 = t3[:, j:j + 1, 0:128]
                        lh2 = t3[:, j:j + 1, 128:256]
                        nc.tensor.matmul(ps1[:, 0:BSTRIDE], lh1, rhs,
                                         start=first, stop=last,
                                         skip_group_check=True)
                        nc.tensor.matmul(ps2[:, 0:BSTRIDE], lh2, rhs,
                                         start=first, stop=last,
                                         skip_group_check=True)
                so = soutp.tile([128, 514], f32)
                nc.vector.tensor_copy(out=so[:, 0:257], in_=ps1[:, 0:257])
                nc.vector.tensor_copy(out=so[:, 257:514], in_=ps2[:, 0:257])
                # ACT-issued DMA: keeps the SP queue free to prefetch the
                # next sample's tiles (no head-of-line blocking on DVE)
                nc.scalar.dma_start(
                    out=stats[s].rearrange("t p c -> p t c"),
                    in_=so[:].rearrange("p (t c) -> p t c", c=257),
                )
    _split_sync_waits(nc)
    return nc


def _build_pass2():
    import concourse.bass as bass
    import concourse.mybir as mybir

    f32 = mybir.dt.float32
    alu = mybir.AluOpType
    T = 7  # tiles per sample: 8 output rows (16 input rows) each

    nc = bass.Bass("TRN2", target_bir_lowering=False, debug=False,
                   num_devices=N_CORES)
    # Host pre-transposes the input to k-planes so every DVE read is
    # contiguous and one DMA loads a whole tile:
    #   x2[s, t, p, tp*2048 + k*512 + o*256 + q*64 + j]
    # with p = (ohi, wih) a pair of output pixels per half-tile tp,
    # o the pixel slot, q = 2*di+dj the input quarter, j the channel group.
    x = nc.dram_tensor("x", [SPC, T, 112, 4096], f32,
                       kind="ExternalInput").ap()
    wb = nc.dram_tensor("wb", [SPC, 128, 8], f32, kind="ExternalInput").ap()
    # out[s, t, tp, ohi, wih, o, c'] row-major == out rows 8t+4tp+ohi
    out = nc.dram_tensor("out", [SPC, T, 2, 4, 28, 2, C], f32,
                         kind="ExternalOutput").ap()

    # Whole chain on DVE: same-engine deps cost no semaphore waits; DMAs
    # out on the ACT queue so SP only issues loads (no head-of-line stall).
    with _make_tile_context(nc) as tc:
        with (
            tc.tile_pool(name="w", bufs=1) as wpool,
            tc.tile_pool(name="inp", bufs=3) as inp,
            tc.tile_pool(name="acc", bufs=2) as accp,
        ):
            for s in range(SPC):
                wt = wpool.tile([128, 8], f32, tag=f"wb{s}")
                nc.sync.dma_start(out=wt[:], in_=wb[s])
                # touch wb on DVE so later DVE ops inherit the dep by
                # program order instead of each carrying a sem wait
                wl = wpool.tile([128, 8], f32, tag=f"wl{s}")
                nc.vector.tensor_copy(out=wl[:], in_=wt[:])
                w0 = wl[0:112, 0:1]
                w1 = wl[0:112, 1:2]
                w2 = wl[0:112, 2:3]
                w3 = wl[0:112, 3:4]
                bias = wl[0:112, 4:5]
                for t in range(T):
                    it = inp.tile([112, 4096], f32)
                    nc.sync.dma_start(out=it[:], in_=x[s, t])
                    it4 = it[:].rearrange("p (tp k f) -> p tp k f",
                                          tp=2, k=4)
                    a0 = accp.tile([112, 1024], f32, tag="a0")
                    a1 = accp.tile([112, 1024], f32, tag="a1")
                    a2 = accp.tile([112, 1024], f32, tag="a2")
                    ot = accp.tile([112, 1024], f32, tag="ot")

                    def v(tt):
                        return tt[:].rearrange("p (tp f) -> p tp f",
                                               tp=2)[:, :, None, :]

                    # a0 = x0*w0 + bias; a_k = x_k*w_k + a_{k-1}
                    nc.vector.tensor_scalar(
                        v(a0), it4[:, :, 0:1, :], w0, bias,
                        op0=alu.mult, op1=alu.add)
                    nc.vector.scalar_tensor_tensor(
                        v(a1), it4[:, :, 1:2, :], w1, v(a0),
                        op0=alu.mult, op1=alu.add)
                    nc.vector.scalar_tensor_tensor(
                        v(a2), it4[:, :, 2:3, :], w2, v(a1),
                        op0=alu.mult, op1=alu.add)
                    nc.vector.scalar_tensor_tensor(
                        v(ot), it4[:, :, 3:4, :], w3, v(a2),
                        op0=alu.mult, op1=alu.add)
                    ot3 = ot[:].rearrange("p (tp f) -> p tp f", tp=2)
                    for tp in range(2):
                        nc.scalar.dma_start(
                            out=out[s, t, tp].rearrange(
                                "ohi wih o c -> ohi wih (o c)"),
                            in_=ot3[:, tp],
                        )
    _split_sync_waits(nc)
    return nc


def _get_programs():
    global _programs
    if _programs is None:
        _programs = (_build_pass1(), _build_pass2())
    return _programs


def _host_middle(stats):
    """stats: [B, 2, 128, 257] f32 -> w [B, 4] f64, bias [B] f64.

    Follows the reference downstream exactly: gram from (S - N mu mu^T) /
    (sigma sigma^T), comp = eigh(gram f32) top eigenvector on CPU jax.
    """
    stats = stats.astype(np.float64)
    M = np.concatenate([stats[:, 0, :, :256], stats[:, 1, :, :256]], axis=1)
    chansum = np.concatenate([stats[:, 0, :, 256], stats[:, 1, :, 256]], axis=1)

    # fold channels c = 4g+k into columns k
    Mg = M.reshape(B, 64, 4, 64, 4)
    S = np.einsum("bgkgl->bkl", Mg)                      # [B, 4, 4]
    colsum = chansum.reshape(B, 64, 4).sum(axis=1)       # [B, 4]

    mu = colsum / NROWS
    e2 = np.einsum("bkk->bk", S) / NROWS
    var = np.maximum(e2 - mu * mu, 0.0)
    sigma = np.sqrt(var)
    denom = sigma[:, :, None] * sigma[:, None, :]
    gram = (S - NROWS * mu[:, :, None] * mu[:, None, :])
    with np.errstate(divide="ignore", invalid="ignore"):
        gram = np.where(denom > 0, gram / np.where(denom > 0, denom, 1.0), 0.0)

    # eigh with the same implementation/backend the reference uses (CPU jax)
    import jax
    import jax.numpy as jnp
    with jax.default_device(jax.devices("cpu")[0]):
        V = np.asarray(jnp.linalg.eigh(jnp.asarray(gram, jnp.float32))[1])
    comp = V[:, :, -1].astype(np.float64)                # top eigenvector

    with np.errstate(divide="ignore", invalid="ignore"):
        w = np.where(sigma > 0, comp / np.where(sigma > 0, sigma, 1.0), 0.0)
    bias = -(mu * w).sum(axis=1)
    return w, bias


def kernel(x):
    from concourse.bass_utils import run_bass_kernel_spmd

    x = np.ascontiguousarray(np.asarray(x), dtype=np.float32)
    assert x.shape == (B, H, W, C), x.shape
    nc1, nc2 = _get_programs()
    core_ids = list(range(N_CORES))

    shards = [x[c * SPC:(c + 1) * SPC] for c in range(N_CORES)]

    # pass-1 input: data blocks padded to 258 cols with a ones column at 256
    xp = np.zeros((B, BT, BLK_PER_BT, 128, BSTRIDE), np.float32)
    xp[..., :C] = x.reshape(B, BT, BLK_PER_BT, 128, C)
    xp[..., C] = 1.0
    in1 = [{"x": xp[c * SPC:(c + 1) * SPC]} for c in range(N_CORES)]
    kw1 = dict(trace=True, tmpdir=TRACE_DIRS.get("pass1")) if TRACE else {}
    r1 = run_bass_kernel_spmd(nc1, in1, core_ids, **kw1)
    if TRACE:
        LAST_PROFILE["pass1_ns"] = r1.exec_time_ns
    stats = np.concatenate([r1.results[c]["stats"] for c in range(N_CORES)])

    w, bias = _host_middle(stats)
    wbs = []
    for c in range(N_CORES):
        a = np.zeros((SPC, 128, 8), np.float32)
        for s in range(SPC):
            b = c * SPC + s
            a[s, :, 0:4] = w[b].astype(np.float32)
            a[s, :, 4] = np.float32(bias[b])
        wbs.append(a)

    # pass-2 input: k-plane transpose so device reads are contiguous.
    # x[s, h, w, c] with h = 16*t + 8*tp + 2*ohi + di, w = 4*wih + 2*o + dj,
    # c = 4*j + k  ->  x2[s, t, (ohi, wih), (tp, k, o, di, dj, j)]
    xs = x.reshape(B, 7, 2, 4, 2, 28, 2, 2, 64, 4)
    xt = np.ascontiguousarray(xs.transpose(0, 1, 3, 5, 2, 9, 6, 4, 7, 8))
    x2h = xt.reshape(B, 7, 112, 4096)
    in2 = [{"x": x2h[c * SPC:(c + 1) * SPC], "wb": wbs[c]}
           for c in range(N_CORES)]
    kw2 = dict(trace=True, tmpdir=TRACE_DIRS.get("pass2")) if TRACE else {}
    r2 = run_bass_kernel_spmd(nc2, in2, core_ids, **kw2)
    if TRACE:
        LAST_PROFILE["pass2_ns"] = r2.exec_time_ns

    out = np.concatenate([r2.results[c]["out"] for c in range(N_CORES)])
    # [B, 7, 2, 4, 28, 2, C]: rows (t, tp, ohi) and cols (wih, o) are both
    # row-major -> plain reshape recovers [B, 56, 56, C]
    return out.reshape(B, HO, WO, C)


# revision 27
# speedup vs baseline: 1.9004x; 1.3616x over previous
"""BPCA pooling layer on 8 Trainium2 NeuronCores (Bass/Tile).

Math: per sample, the reference's `data = patches.reshape(-1, 4)` groups 4
consecutive channels (C=256 is divisible by 4), so `data` is exactly the
sample's contiguous buffer viewed as [N, 4] with N = H*W*C/4.  The layer is:

  1. per-column mean/std over N rows, dn = (data-mean)/std
  2. gram = dn^T dn (4x4), comp = top eigenvector (jnp.linalg.eigh)
  3. out = (dn @ comp) reshaped to [H/2, W/2, C] with channel permutation
     c' = (2*di+dj)*64 + (c//4)

Device plan (2 samples per core, pure data parallel):
  pass 1: PE computes the full 256x256 channel second-moment matrix
          M[c,c'] = sum_pix x[pix,c]*x[pix,c'] plus channel sums (ones
          column), accumulated in PSUM over all pixels.  float32r matmuls
          (1 col/cycle at N>=256) keep PE under the DMA roofline.
  host:   fold M into the 4x4 gram (S_kl = sum_g M[4g+k,4g+l]), compute
          mean/std/gram in f64, eigh on CPU jax (same implementation the
          reference uses), derive w_k = comp_k/std_k and
          bias = -sum_k mean_k*comp_k/std_k.
  pass 2: out = sum_k x_k*w_k + bias -- 4 fused scalar_tensor_tensor /
          activation ops per tile on DVE+ACT, streaming at the DMA roofline,
          with the output channel permutation folded into the DMA pattern.
"""

import numpy as np

# ---------------------------------------------------------------------------
# Problem constants (hardcoded per spec)
# ---------------------------------------------------------------------------
B, H, W, C = 16, 112, 112, 256
N_CORES = 8
SPC = B // N_CORES          # samples per core = 2
PIX = H * W                 # 12544 pixels per sample
NBLK = PIX // 128           # 98 pixel-blocks of 128
BT = 7                      # pass-1 big tiles per sample
BLK_PER_BT = NBLK // BT     # 14 blocks per big tile
BSTRIDE = 258               # per-block SBUF cols: 256 data + 1 ones + 1 pad
NROWS = PIX * C // 4        # 802816 rows of the [N, 4] data matrix
HO, WO = H // 2, W // 2     # 56 x 56 output
T2 = 14                     # pass-2 tiles per sample (4 output rows each)
HPT = HO // T2              # 4 output rows per pass-2 tile

_programs = None
LAST_PROFILE = {}
TRACE = False
TRACE_DIRS = {}


# ---------------------------------------------------------------------------
# TileContext with a walrus-compatible tail drain
# ---------------------------------------------------------------------------
def _make_tile_context(nc):
    from concourse.tile import TileContext
    return TileContext(nc)


def _split_sync_waits(nc):
    """walrus (CoreV2/V3 codegen) rejects instructions carrying more than 2
    sync commands (waits + updates combined); Tile freely emits e.g. 2 waits
    + 1 update.  Hoist excess waits onto same-engine NOPs inserted directly
    before the offending instruction -- same engine means the same program-
    order point, so semantics are unchanged."""
    import concourse.mybir as mybir

    def mint_nop(engine):
        inner = nc.engines[engine].nop().ins
        for blk in nc.m.functions[0].blocks:
            il = blk.instructions
            for k in range(len(il) - 1, -1, -1):
                if il[k] is inner:
                    il.pop(k)
                    return inner
        raise RuntimeError("minted nop not found in any block")

    for fn in nc.m.functions:
        for blk in fn.blocks:
            il = blk.instructions
            i = 0
            while i < len(il):
                inst = il[i]
                si = inst.sync_info
                waits = list(si.on_wait) if si and si.on_wait else []
                upds = list(si.on_update) if si and si.on_update else []
                # observed walrus limits: at most 1 wait per instruction
                # (1 wait + 1 update compiles; 2 waits anywhere does not)
                if len(waits) > 1:
                    extra, keep = waits[:-1], waits[-1:]
                    for wchunk in extra:
                        nop = mint_nop(inst.engine)
                        nop.sync_info = mybir.SyncInfo(
                            on_wait=[wchunk], on_update=[])
                        il.insert(i, nop)
                        i += 1
                    inst.sync_info = mybir.SyncInfo(
                        on_wait=keep, on_update=upds)
                i += 1


def _build_pass1():
    import concourse.bass as bass
    import concourse.mybir as mybir

    f32 = mybir.dt.float32
    f32r = mybir.dt.float32r

    nc = bass.Bass("TRN2", target_bir_lowering=False, debug=False,
                   num_devices=N_CORES)
    # float32r: same bits as f32 (np dtype float32); typing the whole
    # producer chain f32r satisfies walrus's checkMatmultFP32r while the
    # PE runs the matmuls at 1 col/cycle (vs 4 for plain fp32).
    # The host pre-interleaves a ones column per block (col 256 of each
    # 258-wide block) so one DMA loads data + ones and no on-device memset
    # is needed.
    x = nc.dram_tensor("x", [SPC, BT, 128, BLK_PER_BT * BSTRIDE], f32r,
                       kind="ExternalInput").ap()
    stats = nc.dram_tensor("stats", [SPC, 2, 128, 257], f32,
                           kind="ExternalOutput").ap()

    with _make_tile_context(nc) as tc:
        with (
            tc.tile_pool(name="inp", bufs=3) as inp,
            tc.tile_pool(name="psum", bufs=2, space="PSUM") as psum,
            tc.tile_pool(name="sout", bufs=2) as soutp,
        ):
            for s in range(SPC):
                # fp32r matmuls need an even moving free size -> 258
                ps1 = psum.tile([128, BSTRIDE], f32, tag="ps1")
                ps2 = psum.tile([128, BSTRIDE], f32, tag="ps2")
                for bt in range(BT):
                    t = inp.tile([128, BLK_PER_BT * BSTRIDE], f32r)
                    t3 = t[:].rearrange("p (j b) -> p j b", b=BSTRIDE)
                    # host layout matches SBUF exactly: 128-partition fully
                    # contiguous DMA (~420 GB/s vs ~250 at 112 partitions)
                    nc.sync.dma_start(out=t[:], in_=x[s, bt])
                    for j in range(BLK_PER_BT):
                        first = bt == 0 and j == 0
                        last = bt == BT - 1 and j == BLK_PER_BT - 1
                        rhs = t3[:, j:j + 1, 0:BSTRIDE]
                        lh1 = t3[:, j:j + 1, 0:128]
                        lh2 = t3[:, j:j + 1, 128:256]
                        nc.tensor.matmul(ps1[:, 0:BSTRIDE], lh1, rhs,
                                         start=first, stop=last,
                                         skip_group_check=True)
                        nc.tensor.matmul(ps2[:, 0:BSTRIDE], lh2, rhs,
                                         start=first, stop=last,
                                         skip_group_check=True)
                so = soutp.tile([128, 514], f32)
                nc.vector.tensor_copy(out=so[:, 0:257], in_=ps1[:, 0:257])
                nc.vector.tensor_copy(out=so[:, 257:514], in_=ps2[:, 0:257])
                # ACT-issued DMA: keeps the SP queue free to prefetch the
                # next sample's tiles (no head-of-line blocking on DVE)
                nc.scalar.dma_start(
                    out=stats[s].rearrange("t p c -> p t c"),
                    in_=so[:].rearrange("p (t c) -> p t c", c=257),
                )
    _split_sync_waits(nc)
    return nc


def _build_pass2():
    import concourse.bass as bass
    import concourse.mybir as mybir

    f32 = mybir.dt.float32
    alu = mybir.AluOpType
    T = 7   # tiles; each covers 7 output pixels per partition
    OPP = 7 * C  # free elems per (partition, k-plane) per tile

    nc = bass.Bass("TRN2", target_bir_lowering=False, debug=False,
                   num_devices=N_CORES)
    # Partition p = (s_local*64 + p64): each sample's 3136 output pixels
    # split as 64 partitions x 49 outputs, so both samples fill the 128
    # partitions and every DMA is a fully-contiguous 128-partition
    # transfer (measured ~420 GB/s vs ~250 at 112 partitions).
    # Host pre-transposes to k-planes: x[t, p, k*OPP + oo*C + c'].
    x = nc.dram_tensor("x", [T, 128, 4 * OPP], f32,
                       kind="ExternalInput").ap()
    # per-partition weights: rows 0..63 sample 0, 64..127 sample 1
    wb = nc.dram_tensor("wb", [128, 8], f32, kind="ExternalInput").ap()
    out = nc.dram_tensor("out", [T, 128, OPP], f32,
                         kind="ExternalOutput").ap()

    with _make_tile_context(nc) as tc:
        with (
            tc.tile_pool(name="w", bufs=1) as wpool,
            tc.tile_pool(name="inp", bufs=3) as inp,
            tc.tile_pool(name="acc", bufs=2) as accp,
        ):
            wt = wpool.tile([128, 8], f32, tag="wb")
            nc.sync.dma_start(out=wt[:], in_=wb[:])
            # touch wb on DVE so later DVE ops inherit the dep by program
            # order instead of each carrying a sem wait
            wl = wpool.tile([128, 8], f32, tag="wl")
            nc.vector.tensor_copy(out=wl[:], in_=wt[:])
            w = [wl[:, k:k + 1] for k in range(4)]
            bias = wl[:, 4:5]
            for t in range(T):
                it = inp.tile([128, 4 * OPP], f32)
                nc.sync.dma_start(out=it[:], in_=x[t])
                a0 = accp.tile([128, OPP], f32, tag="a0")
                a1 = accp.tile([128, OPP], f32, tag="a1")
                a2 = accp.tile([128, OPP], f32, tag="a2")
                ot = accp.tile([128, OPP], f32, tag="ot")
                # a0 = x0*w0 + bias; a_k = x_k*w_k + a_{k-1}; all contiguous
                nc.vector.tensor_scalar(
                    a0[:], it[:, 0:OPP], w[0], bias,
                    op0=alu.mult, op1=alu.add)
                nc.vector.scalar_tensor_tensor(
                    a1[:], it[:, OPP:2 * OPP], w[1], a0[:],
                    op0=alu.mult, op1=alu.add)
                nc.vector.scalar_tensor_tensor(
                    a2[:], it[:, 2 * OPP:3 * OPP], w[2], a1[:],
                    op0=alu.mult, op1=alu.add)
                nc.vector.scalar_tensor_tensor(
                    ot[:], it[:, 3 * OPP:4 * OPP], w[3], a2[:],
                    op0=alu.mult, op1=alu.add)
                # ACT-issued store: SP queue stays free to prefetch loads
                nc.scalar.dma_start(out=out[t], in_=ot[:])
    _split_sync_waits(nc)
    return nc


def _get_programs():
    global _programs
    if _programs is None:
        _programs = (_build_pass1(), _build_pass2())
    return _programs


def _host_middle(stats):
    """stats: [B, 2, 128, 257] f32 -> w [B, 4] f64, bias [B] f64.

    Follows the reference downstream exactly: gram from (S - N mu mu^T) /
    (sigma sigma^T), comp = eigh(gram f32) top eigenvector on CPU jax.
    """
    stats = stats.astype(np.float64)
    M = np.concatenate([stats[:, 0, :, :256], stats[:, 1, :, :256]], axis=1)
    chansum = np.concatenate([stats[:, 0, :, 256], stats[:, 1, :, 256]], axis=1)

    # fold channels c = 4g+k into columns k
    Mg = M.reshape(B, 64, 4, 64, 4)
    S = np.einsum("bgkgl->bkl", Mg)                      # [B, 4, 4]
    colsum = chansum.reshape(B, 64, 4).sum(axis=1)       # [B, 4]

    mu = colsum / NROWS
    e2 = np.einsum("bkk->bk", S) / NROWS
    var = np.maximum(e2 - mu * mu, 0.0)
    sigma = np.sqrt(var)
    denom = sigma[:, :, None] * sigma[:, None, :]
    gram = (S - NROWS * mu[:, :, None] * mu[:, None, :])
    with np.errstate(divide="ignore", invalid="ignore"):
        gram = np.where(denom > 0, gram / np.where(denom > 0, denom, 1.0), 0.0)

    # eigh with the same implementation/backend the reference uses (CPU jax)
    import jax
    import jax.numpy as jnp
    with jax.default_device(jax.devices("cpu")[0]):
        V = np.asarray(jnp.linalg.eigh(jnp.asarray(gram, jnp.float32))[1])
    comp = V[:, :, -1].astype(np.float64)                # top eigenvector

    with np.errstate(divide="ignore", invalid="ignore"):
        w = np.where(sigma > 0, comp / np.where(sigma > 0, sigma, 1.0), 0.0)
    bias = -(mu * w).sum(axis=1)
    return w, bias


def kernel(x):
    from concourse.bass_utils import run_bass_kernel_spmd

    x = np.ascontiguousarray(np.asarray(x), dtype=np.float32)
    assert x.shape == (B, H, W, C), x.shape
    nc1, nc2 = _get_programs()
    core_ids = list(range(N_CORES))

    # pass-1 input: data blocks padded to 258 cols with a ones column at
    # 256, laid out exactly like the SBUF tile ([128 partitions, 14 blocks])
    xp = np.zeros((B, BT, 128, BLK_PER_BT, BSTRIDE), np.float32)
    xp[..., :C] = x.reshape(B, BT, BLK_PER_BT, 128, C).transpose(0, 1, 3, 2, 4)
    xp[..., C] = 1.0
    xp = xp.reshape(B, BT, 128, BLK_PER_BT * BSTRIDE)
    in1 = [{"x": xp[c * SPC:(c + 1) * SPC]} for c in range(N_CORES)]
    kw1 = dict(trace=True, tmpdir=TRACE_DIRS.get("pass1")) if TRACE else {}
    r1 = run_bass_kernel_spmd(nc1, in1, core_ids, **kw1)
    if TRACE:
        LAST_PROFILE["pass1_ns"] = r1.exec_time_ns
    stats = np.concatenate([r1.results[c]["stats"] for c in range(N_CORES)])

    w, bias = _host_middle(stats)
    wbs = []
    for c in range(N_CORES):
        a = np.zeros((128, 8), np.float32)
        for s in range(SPC):
            b = c * SPC + s
            a[s * 64:(s + 1) * 64, 0:4] = w[b].astype(np.float32)
            a[s * 64:(s + 1) * 64, 4] = np.float32(bias[b])
        wbs.append(a)

    # pass-2 input: k-plane transpose, output-pixel-major.
    #   xplanes[s, outpix=(hi*56+wi), k, c'=(2di+dj)*64+j] = x[s,2hi+di,2wi+dj,4j+k]
    # then outpix = p64*49 + t*7 + oo -> x2[s, t, p64, k, oo, c']
    xpl = x.reshape(B, HO, 2, WO, 2, C // 4, 4).transpose(0, 1, 3, 6, 2, 4, 5)
    xpl = np.ascontiguousarray(xpl).reshape(B, 64, 7, 7, 4, C)
    x2h = np.ascontiguousarray(xpl.transpose(0, 2, 1, 4, 3, 5))
    # per core: partitions = [sample0's 64 | sample1's 64]
    in2 = []
    for c in range(N_CORES):
        pair = x2h[c * SPC:(c + 1) * SPC]          # [2, 7, 64, 4, 7, C]
        arr = np.ascontiguousarray(pair.transpose(1, 0, 2, 3, 4, 5))
        in2.append({"x": arr.reshape(7, 128, 4 * 7 * C), "wb": wbs[c]})
    kw2 = dict(trace=True, tmpdir=TRACE_DIRS.get("pass2")) if TRACE else {}
    r2 = run_bass_kernel_spmd(nc2, in2, core_ids, **kw2)
    if TRACE:
        LAST_PROFILE["pass2_ns"] = r2.exec_time_ns

    # gather: out[t, s*64+p64, oo*C+c'] -> [B, HO, WO, C]
    outs = []
    for c in range(N_CORES):
        o = r2.results[c]["out"].reshape(7, 2, 64, 7, C)
        o = o.transpose(1, 2, 0, 3, 4).reshape(SPC, HO, WO, C)
        outs.append(o)
    return np.ascontiguousarray(np.concatenate(outs))


# revision 29
# speedup vs baseline: 2.0244x; 1.0653x over previous
"""BPCA pooling layer on 8 Trainium2 NeuronCores (Bass/Tile).

Math: per sample, the reference's `data = patches.reshape(-1, 4)` groups 4
consecutive channels (C=256 is divisible by 4), so `data` is exactly the
sample's contiguous buffer viewed as [N, 4] with N = H*W*C/4.  The layer is:

  1. per-column mean/std over N rows, dn = (data-mean)/std
  2. gram = dn^T dn (4x4), comp = top eigenvector (jnp.linalg.eigh)
  3. out = (dn @ comp) reshaped to [H/2, W/2, C] with channel permutation
     c' = (2*di+dj)*64 + (c//4)

Device plan (2 samples per core, pure data parallel):
  pass 1: PE computes the full 256x256 channel second-moment matrix
          M[c,c'] = sum_pix x[pix,c]*x[pix,c'] plus channel sums (ones
          column), accumulated in PSUM over all pixels.  float32r matmuls
          (1 col/cycle at N>=256) keep PE under the DMA roofline.
  host:   fold M into the 4x4 gram (S_kl = sum_g M[4g+k,4g+l]), compute
          mean/std/gram in f64, eigh on CPU jax (same implementation the
          reference uses), derive w_k = comp_k/std_k and
          bias = -sum_k mean_k*comp_k/std_k.
  pass 2: out = sum_k x_k*w_k + bias -- 4 fused scalar_tensor_tensor /
          activation ops per tile on DVE+ACT, streaming at the DMA roofline,
          with the output channel permutation folded into the DMA pattern.
"""

import numpy as np

# ---------------------------------------------------------------------------
# Problem constants (hardcoded per spec)
# ---------------------------------------------------------------------------
B, H, W, C = 16, 112, 112, 256
N_CORES = 8
SPC = B // N_CORES          # samples per core = 2
PIX = H * W                 # 12544 pixels per sample
NBLK = PIX // 128           # 98 pixel-blocks of 128
BT = 7                      # pass-1 big tiles per sample
BLK_PER_BT = NBLK // BT     # 14 blocks per big tile
BSTRIDE = 258               # per-block SBUF cols: 256 data + 1 ones + 1 pad
NROWS = PIX * C // 4        # 802816 rows of the [N, 4] data matrix
HO, WO = H // 2, W // 2     # 56 x 56 output
T2 = 14                     # pass-2 tiles per sample (4 output rows each)
HPT = HO // T2              # 4 output rows per pass-2 tile

_programs = None
LAST_PROFILE = {}
TRACE = False
TRACE_DIRS = {}


# ---------------------------------------------------------------------------
# TileContext with a walrus-compatible tail drain
# ---------------------------------------------------------------------------
def _make_tile_context(nc):
    from concourse.tile import TileContext
    return TileContext(nc)


def _split_sync_waits(nc):
    """walrus (CoreV2/V3 codegen) rejects instructions carrying more than 2
    sync commands (waits + updates combined); Tile freely emits e.g. 2 waits
    + 1 update.  Hoist excess waits onto same-engine NOPs inserted directly
    before the offending instruction -- same engine means the same program-
    order point, so semantics are unchanged."""
    import concourse.mybir as mybir

    def mint_nop(engine):
        inner = nc.engines[engine].nop().ins
        for blk in nc.m.functions[0].blocks:
            il = blk.instructions
            for k in range(len(il) - 1, -1, -1):
                if il[k] is inner:
                    il.pop(k)
                    return inner
        raise RuntimeError("minted nop not found in any block")

    for fn in nc.m.functions:
        for blk in fn.blocks:
            il = blk.instructions
            i = 0
            while i < len(il):
                inst = il[i]
                si = inst.sync_info
                waits = list(si.on_wait) if si and si.on_wait else []
                upds = list(si.on_update) if si and si.on_update else []
                # observed walrus limits: at most 1 wait per instruction
                # (1 wait + 1 update compiles; 2 waits anywhere does not)
                if len(waits) > 1:
                    extra, keep = waits[:-1], waits[-1:]
                    for wchunk in extra:
                        nop = mint_nop(inst.engine)
                        nop.sync_info = mybir.SyncInfo(
                            on_wait=[wchunk], on_update=[])
                        il.insert(i, nop)
                        i += 1
                    inst.sync_info = mybir.SyncInfo(
                        on_wait=keep, on_update=upds)
                i += 1


def _build_pass1():
    import concourse.bass as bass
    import concourse.mybir as mybir

    f32 = mybir.dt.float32
    f32r = mybir.dt.float32r

    nc = bass.Bass("TRN2", target_bir_lowering=False, debug=False,
                   num_devices=N_CORES)
    # float32r: same bits as f32 (np dtype float32); typing the whole
    # producer chain f32r satisfies walrus's checkMatmultFP32r while the
    # PE runs the matmuls at 1 col/cycle (vs 4 for plain fp32).
    # The host pre-interleaves a ones column per block (col 256 of each
    # 258-wide block) so one DMA loads data + ones and no on-device memset
    # is needed.
    x = nc.dram_tensor("x", [SPC, 128, NBLK * BSTRIDE], f32r,
                       kind="ExternalInput").ap()
    stats = nc.dram_tensor("stats", [SPC, 2, 128, 257], f32,
                           kind="ExternalOutput").ap()

    with _make_tile_context(nc) as tc:
        with (
            tc.tile_pool(name="inp", bufs=3) as inp,
            tc.tile_pool(name="psum", bufs=2, space="PSUM") as psum,
            tc.tile_pool(name="sout", bufs=2) as soutp,
        ):
            for s in range(SPC):
                # fp32r matmuls need an even moving free size -> 258
                ps1 = psum.tile([128, BSTRIDE], f32, tag="ps1")
                ps2 = psum.tile([128, BSTRIDE], f32, tag="ps2")
                # graduated tile sizes: tiny first tile so the PE starts
                # ~4us earlier instead of waiting on a 1.75MB load
                b0 = 0
                for nb in [2, 12] + [BLK_PER_BT] * 6:
                    t = inp.tile([128, nb * BSTRIDE], f32r, tag="in")
                    t3 = t[:].rearrange("p (j b) -> p j b", b=BSTRIDE)
                    nc.sync.dma_start(
                        out=t[:],
                        in_=x[s, :, b0 * BSTRIDE:(b0 + nb) * BSTRIDE])
                    for j in range(nb):
                        first = b0 + j == 0
                        last = b0 + j == NBLK - 1
                        rhs = t3[:, j:j + 1, 0:BSTRIDE]
                        lh1 = t3[:, j:j + 1, 0:128]
                        lh2 = t3[:, j:j + 1, 128:256]
                        nc.tensor.matmul(ps1[:, 0:BSTRIDE], lh1, rhs,
                                         start=first, stop=last,
                                         skip_group_check=True)
                        nc.tensor.matmul(ps2[:, 0:BSTRIDE], lh2, rhs,
                                         start=first, stop=last,
                                         skip_group_check=True)
                    b0 += nb
                so = soutp.tile([128, 514], f32)
                nc.vector.tensor_copy(out=so[:, 0:257], in_=ps1[:, 0:257])
                nc.vector.tensor_copy(out=so[:, 257:514], in_=ps2[:, 0:257])
                # ACT-issued DMA: keeps the SP queue free to prefetch the
                # next sample's tiles (no head-of-line blocking on DVE)
                nc.scalar.dma_start(
                    out=stats[s].rearrange("t p c -> p t c"),
                    in_=so[:].rearrange("p (t c) -> p t c", c=257),
                )
    _split_sync_waits(nc)
    return nc


def _build_pass2():
    import concourse.bass as bass
    import concourse.mybir as mybir

    f32 = mybir.dt.float32
    alu = mybir.AluOpType
    OO = 49  # output pixels per partition (3136 = 64 partitions x 49)

    nc = bass.Bass("TRN2", target_bir_lowering=False, debug=False,
                   num_devices=N_CORES)
    # Partition p = (s_local*64 + p64): both samples fill 128 partitions so
    # every DMA is a fully-contiguous 128-partition transfer (~420 GB/s).
    # Host pre-transposes to per-tile k-plane blocks:
    #   x[:, off_t + (k*oo_t + oo)*C + c']
    x = nc.dram_tensor("x", [128, OO * 4 * C], f32,
                       kind="ExternalInput").ap()
    wb = nc.dram_tensor("wb", [128, 8], f32, kind="ExternalInput").ap()
    out = nc.dram_tensor("out", [128, OO * C], f32,
                         kind="ExternalOutput").ap()

    with _make_tile_context(nc) as tc:
        with (
            tc.tile_pool(name="w", bufs=1) as wpool,
            tc.tile_pool(name="inp", bufs=3) as inp,
            tc.tile_pool(name="acc", bufs=2) as accp,
        ):
            wt = wpool.tile([128, 8], f32, tag="wb")
            nc.sync.dma_start(out=wt[:], in_=wb[:])
            # touch wb on DVE so later DVE ops inherit the dep by program
            # order instead of each carrying a sem wait
            wl = wpool.tile([128, 8], f32, tag="wl")
            nc.vector.tensor_copy(out=wl[:], in_=wt[:])
            w = [wl[:, k:k + 1] for k in range(4)]
            bias = wl[:, 4:5]
            off = 0
            ooff = 0
            # tiny first tile: DVE starts ~10us earlier
            for oo in [1, 8, 8, 8, 8, 8, 8]:
                F = oo * C
                it = inp.tile([128, 4 * F], f32, tag="it")
                nc.sync.dma_start(out=it[:], in_=x[:, off:off + 4 * F])
                a0 = accp.tile([128, F], f32, tag="a0")
                a1 = accp.tile([128, F], f32, tag="a1")
                a2 = accp.tile([128, F], f32, tag="a2")
                ot = accp.tile([128, F], f32, tag="ot")
                # a0 = x0*w0 + bias; a_k = x_k*w_k + a_{k-1}; all contiguous
                nc.vector.tensor_scalar(
                    a0[:], it[:, 0:F], w[0], bias,
                    op0=alu.mult, op1=alu.add)
                nc.vector.scalar_tensor_tensor(
                    a1[:], it[:, F:2 * F], w[1], a0[:],
                    op0=alu.mult, op1=alu.add)
                nc.vector.scalar_tensor_tensor(
                    a2[:], it[:, 2 * F:3 * F], w[2], a1[:],
                    op0=alu.mult, op1=alu.add)
                nc.vector.scalar_tensor_tensor(
                    ot[:], it[:, 3 * F:4 * F], w[3], a2[:],
                    op0=alu.mult, op1=alu.add)
                # ACT-issued store: SP queue stays free to prefetch loads
                nc.scalar.dma_start(
                    out=out[:, ooff:ooff + F], in_=ot[:])
                off += 4 * F
                ooff += F
    _split_sync_waits(nc)
    return nc


def _get_programs():
    global _programs
    if _programs is None:
        _programs = (_build_pass1(), _build_pass2())
    return _programs


def _host_middle(stats):
    """stats: [B, 2, 128, 257] f32 -> w [B, 4] f64, bias [B] f64.

    Follows the reference downstream exactly: gram from (S - N mu mu^T) /
    (sigma sigma^T), comp = eigh(gram f32) top eigenvector on CPU jax.
    """
    stats = stats.astype(np.float64)
    M = np.concatenate([stats[:, 0, :, :256], stats[:, 1, :, :256]], axis=1)
    chansum = np.concatenate([stats[:, 0, :, 256], stats[:, 1, :, 256]], axis=1)

    # fold channels c = 4g+k into columns k
    Mg = M.reshape(B, 64, 4, 64, 4)
    S = np.einsum("bgkgl->bkl", Mg)                      # [B, 4, 4]
    colsum = chansum.reshape(B, 64, 4).sum(axis=1)       # [B, 4]

    mu = colsum / NROWS
    e2 = np.einsum("bkk->bk", S) / NROWS
    var = np.maximum(e2 - mu * mu, 0.0)
    sigma = np.sqrt(var)
    denom = sigma[:, :, None] * sigma[:, None, :]
    gram = (S - NROWS * mu[:, :, None] * mu[:, None, :])
    with np.errstate(divide="ignore", invalid="ignore"):
        gram = np.where(denom > 0, gram / np.where(denom > 0, denom, 1.0), 0.0)

    # eigh with the same implementation/backend the reference uses (CPU jax)
    import jax
    import jax.numpy as jnp
    with jax.default_device(jax.devices("cpu")[0]):
        V = np.asarray(jnp.linalg.eigh(jnp.asarray(gram, jnp.float32))[1])
    comp = V[:, :, -1].astype(np.float64)                # top eigenvector

    with np.errstate(divide="ignore", invalid="ignore"):
        w = np.where(sigma > 0, comp / np.where(sigma > 0, sigma, 1.0), 0.0)
    bias = -(mu * w).sum(axis=1)
    return w, bias


def kernel(x):
    from concourse.bass_utils import run_bass_kernel_spmd

    x = np.ascontiguousarray(np.asarray(x), dtype=np.float32)
    assert x.shape == (B, H, W, C), x.shape
    nc1, nc2 = _get_programs()
    core_ids = list(range(N_CORES))

    # pass-1 input: data blocks padded to 258 cols with a ones column at
    # 256, laid out exactly like the SBUF tiles ([128 partitions, blocks])
    xp = np.zeros((B, 128, NBLK, BSTRIDE), np.float32)
    xp[..., :C] = x.reshape(B, NBLK, 128, C).transpose(0, 2, 1, 3)
    xp[..., C] = 1.0
    xp = xp.reshape(B, 128, NBLK * BSTRIDE)
    in1 = [{"x": xp[c * SPC:(c + 1) * SPC]} for c in range(N_CORES)]
    kw1 = dict(trace=True, tmpdir=TRACE_DIRS.get("pass1")) if TRACE else {}
    r1 = run_bass_kernel_spmd(nc1, in1, core_ids, **kw1)
    if TRACE:
        LAST_PROFILE["pass1_ns"] = r1.exec_time_ns
    stats = np.concatenate([r1.results[c]["stats"] for c in range(N_CORES)])

    w, bias = _host_middle(stats)
    wbs = []
    for c in range(N_CORES):
        a = np.zeros((128, 8), np.float32)
        for s in range(SPC):
            b = c * SPC + s
            a[s * 64:(s + 1) * 64, 0:4] = w[b].astype(np.float32)
            a[s * 64:(s + 1) * 64, 4] = np.float32(bias[b])
        wbs.append(a)

    # pass-2 input: k-plane transpose, output-pixel-major.
    #   xplanes[s, outpix=(hi*56+wi), k, c'=(2di+dj)*64+j] = x[s,2hi+di,2wi+dj,4j+k]
    # outpix = p64*49 + oo; per tile t (oo block) the free layout is
    # [k, oo_t, c'], tiles concatenated along the free axis
    xpl = x.reshape(B, HO, 2, WO, 2, C // 4, 4).transpose(0, 1, 3, 6, 2, 4, 5)
    xpl = np.ascontiguousarray(xpl).reshape(B, 64, 49, 4, C)
    segs = []
    oo0 = 0
    for oo in [1, 8, 8, 8, 8, 8, 8]:
        seg = xpl[:, :, oo0:oo0 + oo].transpose(0, 1, 3, 2, 4)
        segs.append(seg.reshape(B, 64, 4 * oo * C))
        oo0 += oo
    x2h = np.concatenate(segs, axis=2)             # [B, 64, 49*4*C]
    in2 = []
    for c in range(N_CORES):
        pair = x2h[c * SPC:(c + 1) * SPC]          # [2, 64, 49*4*C]
        in2.append({"x": pair.reshape(128, 49 * 4 * C), "wb": wbs[c]})
    kw2 = dict(trace=True, tmpdir=TRACE_DIRS.get("pass2")) if TRACE else {}
    r2 = run_bass_kernel_spmd(nc2, in2, core_ids, **kw2)
    if TRACE:
        LAST_PROFILE["pass2_ns"] = r2.exec_time_ns

    # gather: out[s*64+p64, oo*C+c'], outpix = p64*49+oo -> [B, HO, WO, C]
    outs = [r2.results[c]["out"].reshape(SPC, HO, WO, C)
            for c in range(N_CORES)]
    return np.ascontiguousarray(np.concatenate(outs))


# revision 30
# speedup vs baseline: 2.1465x; 1.0603x over previous
"""BPCA pooling layer on 8 Trainium2 NeuronCores (Bass/Tile).

Math: per sample, the reference's `data = patches.reshape(-1, 4)` groups 4
consecutive channels (C=256 is divisible by 4), so `data` is exactly the
sample's contiguous buffer viewed as [N, 4] with N = H*W*C/4.  The layer is:

  1. per-column mean/std over N rows, dn = (data-mean)/std
  2. gram = dn^T dn (4x4), comp = top eigenvector (jnp.linalg.eigh)
  3. out = (dn @ comp) reshaped to [H/2, W/2, C] with channel permutation
     c' = (2*di+dj)*64 + (c//4)

Device plan (2 samples per core, pure data parallel):
  pass 1: PE computes the full 256x256 channel second-moment matrix
          M[c,c'] = sum_pix x[pix,c]*x[pix,c'] plus channel sums (ones
          column), accumulated in PSUM over all pixels.  float32r matmuls
          (1 col/cycle at N>=256) keep PE under the DMA roofline.
  host:   fold M into the 4x4 gram (S_kl = sum_g M[4g+k,4g+l]), compute
          mean/std/gram in f64, eigh on CPU jax (same implementation the
          reference uses), derive w_k = comp_k/std_k and
          bias = -sum_k mean_k*comp_k/std_k.
  pass 2: out = sum_k x_k*w_k + bias -- 4 fused scalar_tensor_tensor /
          activation ops per tile on DVE+ACT, streaming at the DMA roofline,
          with the output channel permutation folded into the DMA pattern.
"""

import numpy as np

# ---------------------------------------------------------------------------
# Problem constants (hardcoded per spec)
# ---------------------------------------------------------------------------
B, H, W, C = 16, 112, 112, 256
N_CORES = 8
SPC = B // N_CORES          # samples per core = 2
PIX = H * W                 # 12544 pixels per sample
NBLK = PIX // 128           # 98 pixel-blocks of 128
BT = 7                      # pass-1 big tiles per sample
BLK_PER_BT = NBLK // BT     # 14 blocks per big tile
BSTRIDE = 258               # per-block SBUF cols: 256 data + 1 ones + 1 pad
NROWS = PIX * C // 4        # 802816 rows of the [N, 4] data matrix
HO, WO = H // 2, W // 2     # 56 x 56 output
T2 = 14                     # pass-2 tiles per sample (4 output rows each)
HPT = HO // T2              # 4 output rows per pass-2 tile

_programs = None
LAST_PROFILE = {}
TRACE = False
TRACE_DIRS = {}


# ---------------------------------------------------------------------------
# TileContext with a walrus-compatible tail drain
# ---------------------------------------------------------------------------
def _make_tile_context(nc):
    from concourse.tile import TileContext
    return TileContext(nc)


def _split_sync_waits(nc):
    """walrus (CoreV2/V3 codegen) rejects instructions carrying more than 2
    sync commands (waits + updates combined); Tile freely emits e.g. 2 waits
    + 1 update.  Hoist excess waits onto same-engine NOPs inserted directly
    before the offending instruction -- same engine means the same program-
    order point, so semantics are unchanged."""
    import concourse.mybir as mybir

    def mint_nop(engine):
        inner = nc.engines[engine].nop().ins
        for blk in nc.m.functions[0].blocks:
            il = blk.instructions
            for k in range(len(il) - 1, -1, -1):
                if il[k] is inner:
                    il.pop(k)
                    return inner
        raise RuntimeError("minted nop not found in any block")

    for fn in nc.m.functions:
        for blk in fn.blocks:
            il = blk.instructions
            i = 0
            while i < len(il):
                inst = il[i]
                si = inst.sync_info
                waits = list(si.on_wait) if si and si.on_wait else []
                upds = list(si.on_update) if si and si.on_update else []
                # observed walrus limits: at most 1 wait per instruction
                # (1 wait + 1 update compiles; 2 waits anywhere does not)
                if len(waits) > 1:
                    extra, keep = waits[:-1], waits[-1:]
                    for wchunk in extra:
                        nop = mint_nop(inst.engine)
                        nop.sync_info = mybir.SyncInfo(
                            on_wait=[wchunk], on_update=[])
                        il.insert(i, nop)
                        i += 1
                    inst.sync_info = mybir.SyncInfo(
                        on_wait=keep, on_update=upds)
                i += 1


def _build_pass1():
    import concourse.bass as bass
    import concourse.mybir as mybir

    f32 = mybir.dt.float32
    f32r = mybir.dt.float32r

    nc = bass.Bass("TRN2", target_bir_lowering=False, debug=False,
                   num_devices=N_CORES)
    # float32r: same bits as f32 (np dtype float32); typing the whole
    # producer chain f32r satisfies walrus's checkMatmultFP32r while the
    # PE runs the matmuls at 1 col/cycle (vs 4 for plain fp32).
    # The host pre-interleaves a ones column per block (col 256 of each
    # 258-wide block) so one DMA loads data + ones and no on-device memset
    # is needed.
    x = nc.dram_tensor("x", [SPC, 128, NBLK * BSTRIDE], f32r,
                       kind="ExternalInput").ap()
    stats = nc.dram_tensor("stats", [SPC, 2, 128, 257], f32,
                           kind="ExternalOutput").ap()

    with _make_tile_context(nc) as tc:
        with (
            tc.tile_pool(name="inp", bufs=4) as inp,
            tc.tile_pool(name="psum", bufs=2, space="PSUM") as psum,
            tc.tile_pool(name="sout", bufs=2) as soutp,
        ):
            for s in range(SPC):
                # fp32r matmuls need an even moving free size -> 258
                ps1 = psum.tile([128, BSTRIDE], f32, tag="ps1")
                ps2 = psum.tile([128, BSTRIDE], f32, tag="ps2")
                # graduated tile sizes: tiny first tile so the PE starts
                # ~4us earlier instead of waiting on a 1.75MB load
                b0 = 0
                for nb in [2, 12] + [BLK_PER_BT] * 6:
                    t = inp.tile([128, nb * BSTRIDE], f32r, tag="in")
                    t3 = t[:].rearrange("p (j b) -> p j b", b=BSTRIDE)
                    nc.sync.dma_start(
                        out=t[:],
                        in_=x[s, :, b0 * BSTRIDE:(b0 + nb) * BSTRIDE])
                    for j in range(nb):
                        first = b0 + j == 0
                        last = b0 + j == NBLK - 1
                        rhs = t3[:, j:j + 1, 0:BSTRIDE]
                        lh1 = t3[:, j:j + 1, 0:128]
                        lh2 = t3[:, j:j + 1, 128:256]
                        nc.tensor.matmul(ps1[:, 0:BSTRIDE], lh1, rhs,
                                         start=first, stop=last,
                                         skip_group_check=True)
                        nc.tensor.matmul(ps2[:, 0:BSTRIDE], lh2, rhs,
                                         start=first, stop=last,
                                         skip_group_check=True)
                    b0 += nb
                so = soutp.tile([128, 514], f32)
                nc.vector.tensor_copy(out=so[:, 0:257], in_=ps1[:, 0:257])
                nc.vector.tensor_copy(out=so[:, 257:514], in_=ps2[:, 0:257])
                # ACT-issued DMA: keeps the SP queue free to prefetch the
                # next sample's tiles (no head-of-line blocking on DVE)
                nc.scalar.dma_start(
                    out=stats[s].rearrange("t p c -> p t c"),
                    in_=so[:].rearrange("p (t c) -> p t c", c=257),
                )
    _split_sync_waits(nc)
    return nc


def _build_pass2():
    import concourse.bass as bass
    import concourse.mybir as mybir

    f32 = mybir.dt.float32
    alu = mybir.AluOpType
    OO = 49  # output pixels per partition (3136 = 64 partitions x 49)

    nc = bass.Bass("TRN2", target_bir_lowering=False, debug=False,
                   num_devices=N_CORES)
    # Partition p = (s_local*64 + p64): both samples fill 128 partitions so
    # every DMA is a fully-contiguous 128-partition transfer (~420 GB/s).
    # Host pre-transposes to per-tile k-plane blocks:
    #   x[:, off_t + (k*oo_t + oo)*C + c']
    x = nc.dram_tensor("x", [128, OO * 4 * C], f32,
                       kind="ExternalInput").ap()
    wb = nc.dram_tensor("wb", [128, 8], f32, kind="ExternalInput").ap()
    out = nc.dram_tensor("out", [128, OO * C], f32,
                         kind="ExternalOutput").ap()

    with _make_tile_context(nc) as tc:
        with (
            tc.tile_pool(name="w", bufs=1) as wpool,
            tc.tile_pool(name="inp", bufs=3) as inp,
            tc.tile_pool(name="acc", bufs=2) as accp,
        ):
            wt = wpool.tile([128, 8], f32, tag="wb")
            nc.sync.dma_start(out=wt[:], in_=wb[:])
            # touch wb on DVE so later DVE ops inherit the dep by program
            # order instead of each carrying a sem wait
            wl = wpool.tile([128, 8], f32, tag="wl")
            nc.vector.tensor_copy(out=wl[:], in_=wt[:])
            w = [wl[:, k:k + 1] for k in range(4)]
            bias = wl[:, 4:5]
            off = 0
            ooff = 0
            # tiny first tile: DVE starts ~10us earlier
            for oo in [1, 8, 8, 8, 8, 8, 8]:
                F = oo * C
                it = inp.tile([128, 4 * F], f32, tag="it")
                nc.sync.dma_start(out=it[:], in_=x[:, off:off + 4 * F])
                a0 = accp.tile([128, F], f32, tag="a0")
                a1 = accp.tile([128, F], f32, tag="a1")
                a2 = accp.tile([128, F], f32, tag="a2")
                ot = accp.tile([128, F], f32, tag="ot")
                # a0 = x0*w0 + bias; a_k = x_k*w_k + a_{k-1}; all contiguous
                nc.vector.tensor_scalar(
                    a0[:], it[:, 0:F], w[0], bias,
                    op0=alu.mult, op1=alu.add)
                nc.vector.scalar_tensor_tensor(
                    a1[:], it[:, F:2 * F], w[1], a0[:],
                    op0=alu.mult, op1=alu.add)
                nc.vector.scalar_tensor_tensor(
                    a2[:], it[:, 2 * F:3 * F], w[2], a1[:],
                    op0=alu.mult, op1=alu.add)
                nc.vector.scalar_tensor_tensor(
                    ot[:], it[:, 3 * F:4 * F], w[3], a2[:],
                    op0=alu.mult, op1=alu.add)
                # ACT-issued store: SP queue stays free to prefetch loads
                nc.scalar.dma_start(
                    out=out[:, ooff:ooff + F], in_=ot[:])
                off += 4 * F
                ooff += F
    _split_sync_waits(nc)
    return nc


def _get_programs():
    global _programs
    if _programs is None:
        _programs = (_build_pass1(), _build_pass2())
    return _programs


def _host_middle(stats):
    """stats: [B, 2, 128, 257] f32 -> w [B, 4] f64, bias [B] f64.

    Follows the reference downstream exactly: gram from (S - N mu mu^T) /
    (sigma sigma^T), comp = eigh(gram f32) top eigenvector on CPU jax.
    """
    stats = stats.astype(np.float64)
    M = np.concatenate([stats[:, 0, :, :256], stats[:, 1, :, :256]], axis=1)
    chansum = np.concatenate([stats[:, 0, :, 256], stats[:, 1, :, 256]], axis=1)

    # fold channels c = 4g+k into columns k
    Mg = M.reshape(B, 64, 4, 64, 4)
    S = np.einsum("bgkgl->bkl", Mg)                      # [B, 4, 4]
    colsum = chansum.reshape(B, 64, 4).sum(axis=1)       # [B, 4]

    mu = colsum / NROWS
    e2 = np.einsum("bkk->bk", S) / NROWS
    var = np.maximum(e2 - mu * mu, 0.0)
    sigma = np.sqrt(var)
    denom = sigma[:, :, None] * sigma[:, None, :]
    gram = (S - NROWS * mu[:, :, None] * mu[:, None, :])
    with np.errstate(divide="ignore", invalid="ignore"):
        gram = np.where(denom > 0, gram / np.where(denom > 0, denom, 1.0), 0.0)

    # eigh with the same implementation/backend the reference uses (CPU jax)
    import jax
    import jax.numpy as jnp
    with jax.default_device(jax.devices("cpu")[0]):
        V = np.asarray(jnp.linalg.eigh(jnp.asarray(gram, jnp.float32))[1])
    comp = V[:, :, -1].astype(np.float64)                # top eigenvector

    with np.errstate(divide="ignore", invalid="ignore"):
        w = np.where(sigma > 0, comp / np.where(sigma > 0, sigma, 1.0), 0.0)
    bias = -(mu * w).sum(axis=1)
    return w, bias


def kernel(x):
    from concourse.bass_utils import run_bass_kernel_spmd

    x = np.ascontiguousarray(np.asarray(x), dtype=np.float32)
    assert x.shape == (B, H, W, C), x.shape
    nc1, nc2 = _get_programs()
    core_ids = list(range(N_CORES))

    # pass-1 input: data blocks padded to 258 cols with a ones column at
    # 256, laid out exactly like the SBUF tiles ([128 partitions, blocks])
    xp = np.zeros((B, 128, NBLK, BSTRIDE), np.float32)
    xp[..., :C] = x.reshape(B, NBLK, 128, C).transpose(0, 2, 1, 3)
    xp[..., C] = 1.0
    xp = xp.reshape(B, 128, NBLK * BSTRIDE)
    in1 = [{"x": xp[c * SPC:(c + 1) * SPC]} for c in range(N_CORES)]
    kw1 = dict(trace=True, tmpdir=TRACE_DIRS.get("pass1")) if TRACE else {}
    r1 = run_bass_kernel_spmd(nc1, in1, core_ids, **kw1)
    if TRACE:
        LAST_PROFILE["pass1_ns"] = r1.exec_time_ns
    stats = np.concatenate([r1.results[c]["stats"] for c in range(N_CORES)])

    w, bias = _host_middle(stats)
    wbs = []
    for c in range(N_CORES):
        a = np.zeros((128, 8), np.float32)
        for s in range(SPC):
            b = c * SPC + s
            a[s * 64:(s + 1) * 64, 0:4] = w[b].astype(np.float32)
            a[s * 64:(s + 1) * 64, 4] = np.float32(bias[b])
        wbs.append(a)

    # pass-2 input: k-plane transpose, output-pixel-major.
    #   xplanes[s, outpix=(hi*56+wi), k, c'=(2di+dj)*64+j] = x[s,2hi+di,2wi+dj,4j+k]
    # outpix = p64*49 + oo; per tile t (oo block) the free layout is
    # [k, oo_t, c'], tiles concatenated along the free axis
    xpl = x.reshape(B, HO, 2, WO, 2, C // 4, 4).transpose(0, 1, 3, 6, 2, 4, 5)
    xpl = np.ascontiguousarray(xpl).reshape(B, 64, 49, 4, C)
    segs = []
    oo0 = 0
    for oo in [1, 8, 8, 8, 8, 8, 8]:
        seg = xpl[:, :, oo0:oo0 + oo].transpose(0, 1, 3, 2, 4)
        segs.append(seg.reshape(B, 64, 4 * oo * C))
        oo0 += oo
    x2h = np.concatenate(segs, axis=2)             # [B, 64, 49*4*C]
    in2 = []
    for c in range(N_CORES):
        pair = x2h[c * SPC:(c + 1) * SPC]          # [2, 64, 49*4*C]
        in2.append({"x": pair.reshape(128, 49 * 4 * C), "wb": wbs[c]})
    kw2 = dict(trace=True, tmpdir=TRACE_DIRS.get("pass2")) if TRACE else {}
    r2 = run_bass_kernel_spmd(nc2, in2, core_ids, **kw2)
    if TRACE:
        LAST_PROFILE["pass2_ns"] = r2.exec_time_ns

    # gather: out[s*64+p64, oo*C+c'], outpix = p64*49+oo -> [B, HO, WO, C]
    outs = [r2.results[c]["out"].reshape(SPC, HO, WO, C)
            for c in range(N_CORES)]
    return np.ascontiguousarray(np.concatenate(outs))
